# revision 13
# baseline (speedup 1.0000x reference)
"""Trainium2 Bass kernel for nn_NNSDecoder (gnn_message_passing).

Reference computation (B=16, N=501, D=128, H=4):
    out[b,i,j] = fc3 . relu(fc2^T relu(feat @ fc1 + b1) + b2) + b3
    feat[b,i,j] = [cp_pre[b,i], cp_post[b,i], cd_pre[b,j], cd_post[b,j]]  (4H=16)

Key algebra: compat[b,n,h] = x[b,n] . (Wk[h] Wq[h]^T q_b), so every
pickup/delivery-side term is linear in h_hat / h_nb rows.  Folding the
head projections and fc1 together gives per-batch maps
    A[b] = h_hat[b] @ G_A1 + h_nb[b] @ G_A2          (N x 32, row/i term)
    C[b] = h_hat[b] @ G_C1 + h_nb[b] @ G_C2          (N x 32, col/j term)
    out[b,i,j] = w3 . relu(W2^T relu(A[b,i] + C[b,j] + b1) + b2) + b3
A and C are tiny (N x 32) and are computed on HOST in fp32; the device
receives crep = (C+b1) replicated 4x across partitions (f16) and
a4[32r+k, t] = A[4t+r, k] (f32), and does only the O(N^2) work.

Per 4-row i-tile t:
    X_t = relu(crep + a4[:,t])            (DVE tensor_scalar, f16, 2x mode)
    Z_t = W2blk @ X_t                     (PE matmul, block-diag f16)
    Y_t = relu(Z_t + b2)                  (ScalarE ACT, f16)
    po  = w3blk @ Y_t                     (PE matmul into packed PSUM)
i-tiles are processed in PAIRS sharing one 2-bank PSUM tile [128,1024]
(NP=512 = exact bank) so Y runs as a single wide op; the stages are
software-pipelined with per-stage skews (X@i, fc2@i-1, Y@i-2, fc3@i-3,
PSUM->SBUF copy@deep skew on DVE) so no engine's in-order queue waits
on another engine's freshest output.  8-tile supergroups drain with one
wide f16 copy + one contiguous raw-dump DMA; the host un-permutes rows.

Sharding: batch dim 16 -> 8 cores x 2 batches (data parallel, weights
replicated). Full inputs in, full output out.
"""

import numpy as np

B, N, D, H = 16, 501, 128, 4
NCORES = 8
BPC = B // NCORES  # batches per core
NP = 512  # padded j: exact PSUM bank (512 f32 = 2KB)
NT = 126  # i-tiles of 4 rows (126*4 = 504 >= 501)
PAIRS = NT // 2  # 63 i-tile pairs per batch
NSG = 16  # supergroups per batch (8 i-tiles / 32 output rows each)

_cache = {}


def _build_program():
    import concourse.bacc as bacc
    import concourse.mybir as mybir
    from concourse.tile import TileContext

    F32 = mybir.dt.float32
    F16 = mybir.dt.float16
    nc = bacc.Bacc("TRN2", target_bir_lowering=False, debug=False, num_devices=1)

    crd = nc.dram_tensor("crd", [BPC, D, NP], F16, kind="ExternalInput")
    a4d = nc.dram_tensor("a4d", [BPC, D, NT], F32, kind="ExternalInput")
    wpd = nc.dram_tensor("wpd", [D, 132], F16, kind="ExternalInput")
    b2r = nc.dram_tensor("b2r", [D, 1], F32, kind="ExternalInput")
    raw = nc.dram_tensor("raw", [BPC, NSG, 100, 2 * NP], F16, kind="ExternalOutput")

    add = mybir.AluOpType.add
    amax = mybir.AluOpType.max
    Relu = mybir.ActivationFunctionType.Relu

    with TileContext(nc) as tc:
        with (
            tc.tile_pool(name="const", bufs=1) as cpool,
            tc.tile_pool(name="batch", bufs=2) as bpool,
            tc.tile_pool(name="x", bufs=8) as xpool,
            tc.tile_pool(name="y", bufs=6) as ypool,
            tc.tile_pool(name="o", bufs=3) as opool,
            tc.tile_pool(name="pz", bufs=2, space="PSUM") as pzpool,
            tc.tile_pool(name="po", bufs=2, space="PSUM") as popool,
        ):
            # inputs for batch 0 first: they gate the first X / fc2
            creps = []
            a4s = []
            for b in range(BPC):
                crep = bpool.tile([D, NP], F16, tag="crep")
                nc.sync.dma_start(crep[:], crd.ap()[b, :, :])
                creps.append(crep)
                a4 = bpool.tile([D, NT], F32, tag="a4")
                nc.gpsimd.dma_start(a4[:], a4d.ap()[b, :, :])
                a4s.append(a4)
                if b == 0:
                    wpt = cpool.tile([D, 132], F16)
                    nc.gpsimd.dma_start(wpt[:], wpd.ap()[:, :])
                    w2t = wpt[:, 0:128]
                    w3t = wpt[:, 128:132]
                    b2t = cpool.tile([D, 1], F32)
                    nc.gpsimd.dma_start(b2t[:], b2r.ap()[:, :])

            for b in range(BPC):
                crep = creps[b]
                a4 = a4s[b]

                # main pair loop, software-pipelined with per-stage skews
                pend_x = {}
                pend_pz = {}
                pend_y = {}
                pend_po = {}
                po2 = None
                for p in range(PAIRS + 3 + 5):
                    if p < PAIRS:
                        t0 = 2 * p
                        xs = []
                        for t in (t0, t0 + 1):
                            x = xpool.tile([D, NP], F16, tag="x")
                            nc.vector.tensor_scalar(
                                out=x[:],
                                in0=crep[:],
                                scalar1=a4[:, t : t + 1],
                                scalar2=0.0,
                                op0=add,
                                op1=amax,
                            )
                            xs.append(x)
                        pend_x[p] = xs

                    pm = p - 1
                    if 0 <= pm < PAIRS:
                        xs = pend_x.pop(pm)
                        pz2 = pzpool.tile([D, 2 * NP], F32, tag="pz")
                        nc.tensor.matmul(
                            pz2[:, 0:NP], w2t, xs[0][:], start=True, stop=True
                        )
                        nc.tensor.matmul(
                            pz2[:, NP : 2 * NP], w2t, xs[1][:], start=True, stop=True
                        )
                        pend_pz[pm] = pz2

                    py = p - 2
                    if 0 <= py < PAIRS:
                        pz2 = pend_pz.pop(py)
                        y2 = ypool.tile([D, 2 * NP], F16, tag="y2")
                        nc.scalar.activation(y2[:], pz2[:], Relu, bias=b2t[:, 0:1])
                        pend_y[py] = y2

                    pf = p - 3
                    if 0 <= pf < PAIRS:
                        s, q = divmod(pf, 4)
                        if q == 0:
                            po2 = popool.tile([D, 2 * NP], F32, tag="po")
                        y2 = pend_y.pop(pf)
                        for j in (0, 1):
                            st = 2 * q + j
                            h, v = st // 4, st % 4
                            nc.tensor.matmul(
                                po2[32 * v : 32 * v + 4, h * NP : h * NP + NP],
                                w3t,
                                y2[:, j * NP : j * NP + NP],
                                start=True,
                                stop=True,
                                tile_position=(0, 32 * v),
                            )
                        if pf == PAIRS - 1 and b == BPC - 1:
                            # very last supergroup: shallow copy + trimmed
                            # split dump so the end-of-kernel drain is short
                            ob2 = opool.tile([D, 2 * NP], F16, tag="ob")
                            nc.vector.tensor_scalar_add(
                                ob2[0:100, :], po2[0:100, :], 0.0
                            )
                            nc.sync.dma_start(
                                raw.ap()[b, s, 0:100, 0:NP], ob2[0:100, 0:NP]
                            )
                            nc.gpsimd.dma_start(
                                raw.ap()[b, s, 0:40, NP : 2 * NP],
                                ob2[0:40, NP : 2 * NP],
                            )
                        elif q == 3 or pf == PAIRS - 1:
                            pend_po[s] = po2

                    # copy/DMA two iters after a supergroup's last fc3, so the
                    # DVE copy never waits mid-queue and never delays X ops
                    pc = p - 8
                    if pc >= 0 and pc % 4 == 0 and (pc // 4) in pend_po:
                        s2 = pc // 4
                        po2c = pend_po.pop(s2)
                        # used fc3 partitions are {32v+r, r<4} (v = slot%4,
                        # max 99); dump them raw, host un-permutes rows.
                        ob2 = opool.tile([D, 2 * NP], F16, tag="ob")
                        nc.vector.tensor_scalar_add(ob2[0:100, :], po2c[0:100, :], 0.0)
                        eng = nc.sync if s2 % 2 == 0 else nc.gpsimd
                        eng.dma_start(raw.ap()[b, s2, :, :], ob2[0:100, :])

    nc.compile()
    return nc


def _host_prep(h_hat, pos_pickup, pos_delivery, solution, Wq1, Wk1, Wq2, Wk2,
               fc1_w, fc1_b):
    """Host-side A/C maps folded with fc1: returns crep (f16) and a4 (f32)."""
    f32 = np.float32
    h_hat = np.asarray(h_hat, f32)
    pp = np.asarray(pos_pickup).astype(np.int64)
    pd = np.asarray(pos_delivery).astype(np.int64)
    sol = np.asarray(solution).astype(np.int64)
    Wq1 = np.asarray(Wq1, f32)
    Wk1 = np.asarray(Wk1, f32)
    Wq2 = np.asarray(Wq2, f32)
    Wk2 = np.asarray(Wk2, f32)
    fc1_w = np.asarray(fc1_w, f32)
    fc1_b = np.asarray(fc1_b, f32)

    crep = np.zeros((B, D, NP), np.float16)
    a4 = np.zeros((B, D, NT), f32)

    for b in range(B):
        hb = h_hat[b]  # (N, D)
        hnb = hb[sol[b]]  # (N, D) gathered neighbours
        p = hb[pp[b]]  # (D,)
        dv = hb[pd[b]]
        # u[h] = Wk[h] @ (Wq[h]^T @ q): compat[n,h] = x[n] . u[h]
        U1p = np.stack([Wk1[h] @ (Wq1[h].T @ p) for h in range(H)], axis=1)
        U2p = np.stack([Wk2[h] @ (Wq2[h].T @ p) for h in range(H)], axis=1)
        U1d = np.stack([Wk1[h] @ (Wq1[h].T @ dv) for h in range(H)], axis=1)
        U2d = np.stack([Wk2[h] @ (Wq2[h].T @ dv) for h in range(H)], axis=1)
        A = hb @ (U1p @ fc1_w[0:4]) + hnb @ (U2p @ fc1_w[4:8])  # (N, 32)
        C = hb @ (U1d @ fc1_w[8:12]) + hnb @ (U2d @ fc1_w[12:16])  # (N, 32)
        Cp = np.zeros((NP, 32), f32)
        Cp[:N] = C
        crep[b] = np.tile((Cp + fc1_b).T.astype(np.float16), (4, 1))
        Ap = np.zeros((4 * NT, 32), f32)
        Ap[:N] = A
        # a4[32r+k, t] = A[4t+r, k]
        a4[b] = Ap.reshape(NT, 4, 32).transpose(1, 2, 0).reshape(D, NT)
    return crep, a4


_last_results = None


def kernel(
    h_hat,
    pos_pickup,
    pos_delivery,
    solution,
    Wq1,
    Wk1,
    Wq2,
    Wk2,
    fc1_w,
    fc1_b,
    fc2_w,
    fc2_b,
    fc3_w,
    fc3_b,
):
    global _last_results
    from concourse.bass_utils import run_bass_kernel_spmd

    f32 = np.float32
    fc2_w = np.asarray(fc2_w, f32)
    fc2_b = np.asarray(fc2_b, f32)
    fc3_w = np.asarray(fc3_w, f32)
    fc3_b = np.asarray(fc3_b, f32)

    crep, a4 = _host_prep(
        h_hat, pos_pickup, pos_delivery, solution, Wq1, Wk1, Wq2, Wk2,
        np.asarray(fc1_w, f32), np.asarray(fc1_b, f32),
    )

    # block-diagonal packed MLP weights (4 independent 32-blocks)
    w2d = np.zeros((D, 128), f32)
    w3d = np.zeros((D, 4), f32)
    for r in range(4):
        w2d[32 * r : 32 * r + 32, 32 * r : 32 * r + 32] = fc2_w
        w3d[32 * r : 32 * r + 32, r : r + 1] = fc3_w.reshape(32, 1)
    b2r = np.tile(fc2_b.reshape(32, 1), (4, 1)).astype(f32)
    wp = np.concatenate([w2d, w3d], axis=1).astype(np.float16)  # [D, 132]

    if "nc" not in _cache:
        _cache["nc"] = _build_program()
    nc = _cache["nc"]

    in_maps = []
    for c in range(NCORES):
        bs = slice(BPC * c, BPC * (c + 1))
        in_maps.append(
            {
                "crd": np.ascontiguousarray(crep[bs]),
                "a4d": np.ascontiguousarray(a4[bs]),
                "wpd": wp,
                "b2r": b2r,
            }
        )

    res = run_bass_kernel_spmd(nc, in_maps, core_ids=list(range(NCORES)))
    _last_results = res

    # un-permute: raw[b, s, 32v+r, 512h+j] holds out row 32s+16h+4v+r
    rows = np.arange(N)
    s_i = rows // 32
    rem = rows % 32
    h_i = rem // 16
    rem2 = rem % 16
    part = 32 * (rem2 // 4) + (rem2 % 4)
    foff = NP * h_i
    cols = np.arange(N)
    out = np.empty((B, N, N), f32)
    for c in range(NCORES):
        rawc = res.results[c]["raw"].astype(f32)  # [BPC, NSG, 100, 2*NP]
        for bb in range(BPC):
            out[BPC * c + bb] = rawc[
                bb, s_i[:, None], part[:, None], foff[:, None] + cols[None, :]
            ]
    b3 = float(fc3_b.reshape(-1)[0])
    if b3 != 0.0:
        out = out + b3
    return out.astype(f32)


# revision 14
# speedup vs baseline: 1.0062x; 1.0062x over previous
"""Trainium2 Bass kernel for nn_NNSDecoder (gnn_message_passing).

Reference computation (B=16, N=501, D=128, H=4):
    out[b,i,j] = fc3 . relu(fc2^T relu(feat @ fc1 + b1) + b2) + b3
    feat[b,i,j] = [cp_pre[b,i], cp_post[b,i], cd_pre[b,j], cd_post[b,j]]  (4H=16)

Key algebra: compat[b,n,h] = x[b,n] . (Wk[h] Wq[h]^T q_b), so every
pickup/delivery-side term is linear in h_hat / h_nb rows.  Folding the
head projections and fc1 together gives per-batch maps
    A[b] = h_hat[b] @ G_A1 + h_nb[b] @ G_A2          (N x 32, row/i term)
    C[b] = h_hat[b] @ G_C1 + h_nb[b] @ G_C2          (N x 32, col/j term)
    out[b,i,j] = w3 . relu(W2^T relu(A[b,i] + C[b,j] + b1) + b2) + b3
A and C are tiny (N x 32) and are computed on HOST in fp32; the device
receives crep = (C+b1) replicated 4x across partitions (f16) and
a4[32r+k, t] = A[4t+r, k] (f32), and does only the O(N^2) work.

Per 4-row i-tile t:
    X_t = relu(crep + a4[:,t])            (DVE tensor_scalar, f16, 2x mode)
    Z_t = W2blk @ X_t                     (PE matmul, block-diag f16)
    Y_t = relu(Z_t + b2)                  (ScalarE ACT, f16)
    po  = w3blk @ Y_t                     (PE matmul into packed PSUM)
i-tiles are processed in PAIRS sharing one 2-bank PSUM tile [128,1024]
(NP=512 = exact bank) so Y runs as a single wide op; the stages are
software-pipelined with per-stage skews (X@i, fc2@i-1, Y@i-2, fc3@i-3,
PSUM->SBUF copy@deep skew on DVE) so no engine's in-order queue waits
on another engine's freshest output.  8-tile supergroups drain with one
wide f16 copy + one contiguous raw-dump DMA; the host un-permutes rows.

Sharding: batch dim 16 -> 8 cores x 2 batches (data parallel, weights
replicated). Full inputs in, full output out.
"""

import numpy as np

B, N, D, H = 16, 501, 128, 4
NCORES = 8
BPC = B // NCORES  # batches per core
NP = 512  # padded j: exact PSUM bank (512 f32 = 2KB)
NT = 126  # i-tiles of 4 rows (126*4 = 504 >= 501)
PAIRS = NT // 2  # 63 i-tile pairs per batch
NSG = 16  # supergroups per batch (8 i-tiles / 32 output rows each)

_cache = {}


def _build_program():
    import concourse.bacc as bacc
    import concourse.mybir as mybir
    from concourse.tile import TileContext

    F32 = mybir.dt.float32
    F16 = mybir.dt.float16
    nc = bacc.Bacc("TRN2", target_bir_lowering=False, debug=False, num_devices=1)

    crd = nc.dram_tensor("crd", [BPC, D, NP], F16, kind="ExternalInput")
    a4d = nc.dram_tensor("a4d", [BPC, D, NT], F32, kind="ExternalInput")
    wpd = nc.dram_tensor("wpd", [D, 132], F16, kind="ExternalInput")
    b2r = nc.dram_tensor("b2r", [D, 1], F32, kind="ExternalInput")
    raw = nc.dram_tensor("raw", [BPC, NSG, 100, 2 * NP], F16, kind="ExternalOutput")

    add = mybir.AluOpType.add
    amax = mybir.AluOpType.max
    Relu = mybir.ActivationFunctionType.Relu

    with TileContext(nc) as tc:
        with (
            tc.tile_pool(name="const", bufs=1) as cpool,
            tc.tile_pool(name="batch", bufs=2) as bpool,
            tc.tile_pool(name="x", bufs=8) as xpool,
            tc.tile_pool(name="y", bufs=6) as ypool,
            tc.tile_pool(name="o", bufs=3) as opool,
            tc.tile_pool(name="pz", bufs=2, space="PSUM") as pzpool,
            tc.tile_pool(name="po", bufs=2, space="PSUM") as popool,
        ):
            # inputs for batch 0 first: they gate the first X / fc2
            creps = []
            a4s = []
            for b in range(BPC):
                crep = bpool.tile([D, NP], F16, tag="crep")
                nc.sync.dma_start(crep[:], crd.ap()[b, :, :])
                creps.append(crep)
                a4 = bpool.tile([D, NT], F32, tag="a4")
                nc.sync.dma_start(a4[:], a4d.ap()[b, :, :])
                a4s.append(a4)
                if b == 0:
                    wpt = cpool.tile([D, 132], F16)
                    nc.sync.dma_start(wpt[:], wpd.ap()[:, :])
                    w2t = wpt[:, 0:128]
                    w3t = wpt[:, 128:132]
                    b2t = cpool.tile([D, 1], F32)
                    nc.sync.dma_start(b2t[:], b2r.ap()[:, :])

            for b in range(BPC):
                crep = creps[b]
                a4 = a4s[b]

                # main pair loop, software-pipelined with per-stage skews
                pend_x = {}
                pend_pz = {}
                pend_y = {}
                pend_po = {}
                po2 = None
                for p in range(PAIRS + 3 + 5):
                    if p < PAIRS:
                        t0 = 2 * p
                        xs = []
                        for t in (t0, t0 + 1):
                            x = xpool.tile([D, NP], F16, tag="x")
                            nc.vector.tensor_scalar(
                                out=x[:],
                                in0=crep[:],
                                scalar1=a4[:, t : t + 1],
                                scalar2=0.0,
                                op0=add,
                                op1=amax,
                            )
                            xs.append(x)
                        pend_x[p] = xs

                    pm = p - 1
                    if 0 <= pm < PAIRS:
                        xs = pend_x.pop(pm)
                        pz2 = pzpool.tile([D, 2 * NP], F32, tag="pz")
                        nc.tensor.matmul(
                            pz2[:, 0:NP], w2t, xs[0][:], start=True, stop=True
                        )
                        nc.tensor.matmul(
                            pz2[:, NP : 2 * NP], w2t, xs[1][:], start=True, stop=True
                        )
                        pend_pz[pm] = pz2

                    py = p - 2
                    if 0 <= py < PAIRS:
                        pz2 = pend_pz.pop(py)
                        y2 = ypool.tile([D, 2 * NP], F16, tag="y2")
                        nc.scalar.activation(y2[:], pz2[:], Relu, bias=b2t[:, 0:1])
                        pend_y[py] = y2

                    pf = p - 3
                    if 0 <= pf < PAIRS:
                        s, q = divmod(pf, 4)
                        if q == 0:
                            po2 = popool.tile([D, 2 * NP], F32, tag="po")
                        y2 = pend_y.pop(pf)
                        for j in (0, 1):
                            st = 2 * q + j
                            h, v = st // 4, st % 4
                            nc.tensor.matmul(
                                po2[32 * v : 32 * v + 4, h * NP : h * NP + NP],
                                w3t,
                                y2[:, j * NP : j * NP + NP],
                                start=True,
                                stop=True,
                                tile_position=(0, 32 * v),
                            )
                        if pf == PAIRS - 1 and b == BPC - 1:
                            # very last supergroup: shallow copy + trimmed
                            # split dump so the end-of-kernel drain is short
                            ob2 = opool.tile([D, 2 * NP], F16, tag="ob")
                            nc.vector.tensor_scalar_add(
                                ob2[0:100, :], po2[0:100, :], 0.0
                            )
                            nc.sync.dma_start(
                                raw.ap()[b, s, 0:100, 0:NP], ob2[0:100, 0:NP]
                            )
                            nc.sync.dma_start(
                                raw.ap()[b, s, 0:40, NP : 2 * NP],
                                ob2[0:40, NP : 2 * NP],
                            )
                        elif q == 3 or pf == PAIRS - 1:
                            pend_po[s] = po2

                    # copy/DMA two iters after a supergroup's last fc3, so the
                    # DVE copy never waits mid-queue and never delays X ops
                    pc = p - 8
                    if pc >= 0 and pc % 4 == 0 and (pc // 4) in pend_po:
                        s2 = pc // 4
                        po2c = pend_po.pop(s2)
                        # used fc3 partitions are {32v+r, r<4} (v = slot%4,
                        # max 99); dump them raw, host un-permutes rows.
                        ob2 = opool.tile([D, 2 * NP], F16, tag="ob")
                        nc.vector.tensor_scalar_add(ob2[0:100, :], po2c[0:100, :], 0.0)
                        nc.sync.dma_start(raw.ap()[b, s2, :, :], ob2[0:100, :])

    nc.compile()
    return nc


def _host_prep(h_hat, pos_pickup, pos_delivery, solution, Wq1, Wk1, Wq2, Wk2,
               fc1_w, fc1_b):
    """Host-side A/C maps folded with fc1: returns crep (f16) and a4 (f32)."""
    f32 = np.float32
    h_hat = np.asarray(h_hat, f32)
    pp = np.asarray(pos_pickup).astype(np.int64)
    pd = np.asarray(pos_delivery).astype(np.int64)
    sol = np.asarray(solution).astype(np.int64)
    Wq1 = np.asarray(Wq1, f32)
    Wk1 = np.asarray(Wk1, f32)
    Wq2 = np.asarray(Wq2, f32)
    Wk2 = np.asarray(Wk2, f32)
    fc1_w = np.asarray(fc1_w, f32)
    fc1_b = np.asarray(fc1_b, f32)

    crep = np.zeros((B, D, NP), np.float16)
    a4 = np.zeros((B, D, NT), f32)

    for b in range(B):
        hb = h_hat[b]  # (N, D)
        hnb = hb[sol[b]]  # (N, D) gathered neighbours
        p = hb[pp[b]]  # (D,)
        dv = hb[pd[b]]
        # u[h] = Wk[h] @ (Wq[h]^T @ q): compat[n,h] = x[n] . u[h]
        U1p = np.stack([Wk1[h] @ (Wq1[h].T @ p) for h in range(H)], axis=1)
        U2p = np.stack([Wk2[h] @ (Wq2[h].T @ p) for h in range(H)], axis=1)
        U1d = np.stack([Wk1[h] @ (Wq1[h].T @ dv) for h in range(H)], axis=1)
        U2d = np.stack([Wk2[h] @ (Wq2[h].T @ dv) for h in range(H)], axis=1)
        A = hb @ (U1p @ fc1_w[0:4]) + hnb @ (U2p @ fc1_w[4:8])  # (N, 32)
        C = hb @ (U1d @ fc1_w[8:12]) + hnb @ (U2d @ fc1_w[12:16])  # (N, 32)
        Cp = np.zeros((NP, 32), f32)
        Cp[:N] = C
        crep[b] = np.tile((Cp + fc1_b).T.astype(np.float16), (4, 1))
        Ap = np.zeros((4 * NT, 32), f32)
        Ap[:N] = A
        # a4[32r+k, t] = A[4t+r, k]
        a4[b] = Ap.reshape(NT, 4, 32).transpose(1, 2, 0).reshape(D, NT)
    return crep, a4


_last_results = None


def kernel(
    h_hat,
    pos_pickup,
    pos_delivery,
    solution,
    Wq1,
    Wk1,
    Wq2,
    Wk2,
    fc1_w,
    fc1_b,
    fc2_w,
    fc2_b,
    fc3_w,
    fc3_b,
):
    global _last_results
    from concourse.bass_utils import run_bass_kernel_spmd

    f32 = np.float32
    fc2_w = np.asarray(fc2_w, f32)
    fc2_b = np.asarray(fc2_b, f32)
    fc3_w = np.asarray(fc3_w, f32)
    fc3_b = np.asarray(fc3_b, f32)

    crep, a4 = _host_prep(
        h_hat, pos_pickup, pos_delivery, solution, Wq1, Wk1, Wq2, Wk2,
        np.asarray(fc1_w, f32), np.asarray(fc1_b, f32),
    )

    # block-diagonal packed MLP weights (4 independent 32-blocks)
    w2d = np.zeros((D, 128), f32)
    w3d = np.zeros((D, 4), f32)
    for r in range(4):
        w2d[32 * r : 32 * r + 32, 32 * r : 32 * r + 32] = fc2_w
        w3d[32 * r : 32 * r + 32, r : r + 1] = fc3_w.reshape(32, 1)
    b2r = np.tile(fc2_b.reshape(32, 1), (4, 1)).astype(f32)
    wp = np.concatenate([w2d, w3d], axis=1).astype(np.float16)  # [D, 132]

    if "nc" not in _cache:
        _cache["nc"] = _build_program()
    nc = _cache["nc"]

    in_maps = []
    for c in range(NCORES):
        bs = slice(BPC * c, BPC * (c + 1))
        in_maps.append(
            {
                "crd": np.ascontiguousarray(crep[bs]),
                "a4d": np.ascontiguousarray(a4[bs]),
                "wpd": wp,
                "b2r": b2r,
            }
        )

    res = run_bass_kernel_spmd(nc, in_maps, core_ids=list(range(NCORES)))
    _last_results = res

    # un-permute: raw[b, s, 32v+r, 512h+j] holds out row 32s+16h+4v+r
    rows = np.arange(N)
    s_i = rows // 32
    rem = rows % 32
    h_i = rem // 16
    rem2 = rem % 16
    part = 32 * (rem2 // 4) + (rem2 % 4)
    foff = NP * h_i
    cols = np.arange(N)
    out = np.empty((B, N, N), f32)
    for c in range(NCORES):
        rawc = res.results[c]["raw"].astype(f32)  # [BPC, NSG, 100, 2*NP]
        for bb in range(BPC):
            out[BPC * c + bb] = rawc[
                bb, s_i[:, None], part[:, None], foff[:, None] + cols[None, :]
            ]
    b3 = float(fc3_b.reshape(-1)[0])
    if b3 != 0.0:
        out = out + b3
    return out.astype(f32)


# revision 15
# speedup vs baseline: 1.0091x; 1.0029x over previous
"""Trainium2 Bass kernel for nn_NNSDecoder (gnn_message_passing).

Reference computation (B=16, N=501, D=128, H=4):
    out[b,i,j] = fc3 . relu(fc2^T relu(feat @ fc1 + b1) + b2) + b3
    feat[b,i,j] = [cp_pre[b,i], cp_post[b,i], cd_pre[b,j], cd_post[b,j]]  (4H=16)

Key algebra: compat[b,n,h] = x[b,n] . (Wk[h] Wq[h]^T q_b), so every
pickup/delivery-side term is linear in h_hat / h_nb rows.  Folding the
head projections and fc1 together gives per-batch maps
    A[b] = h_hat[b] @ G_A1 + h_nb[b] @ G_A2          (N x 32, row/i term)
    C[b] = h_hat[b] @ G_C1 + h_nb[b] @ G_C2          (N x 32, col/j term)
    out[b,i,j] = w3 . relu(W2^T relu(A[b,i] + C[b,j] + b1) + b2) + b3
A and C are tiny (N x 32) and are computed on HOST in fp32; the device
receives crep = (C+b1) replicated 4x across partitions (f16) and
a4[32r+k, t] = A[4t+r, k] (f32), and does only the O(N^2) work.

Per 4-row i-tile t:
    X_t = relu(crep + a4[:,t])            (DVE tensor_scalar, f16, 2x mode)
    Z_t = W2blk @ X_t                     (PE matmul, block-diag f16)
    Y_t = relu(Z_t + b2)                  (ScalarE ACT, f16)
    po  = w3blk @ Y_t                     (PE matmul into packed PSUM)
i-tiles are processed in PAIRS sharing one 2-bank PSUM tile [128,1024]
(NP=512 = exact bank) so Y runs as a single wide op; the stages are
software-pipelined with per-stage skews (X@i, fc2@i-1, Y@i-2, fc3@i-3,
PSUM->SBUF copy@deep skew on DVE) so no engine's in-order queue waits
on another engine's freshest output.  8-tile supergroups drain with one
wide f16 copy + one contiguous raw-dump DMA; the host un-permutes rows.

Sharding: batch dim 16 -> 8 cores x 2 batches (data parallel, weights
replicated). Full inputs in, full output out.
"""

import numpy as np

B, N, D, H = 16, 501, 128, 4
NCORES = 8
BPC = B // NCORES  # batches per core
NP = 512  # padded j: exact PSUM bank (512 f32 = 2KB)
NT = 126  # i-tiles of 4 rows (126*4 = 504 >= 501)
PAIRS = NT // 2  # 63 i-tile pairs per batch
NSG = 16  # supergroups per batch (8 i-tiles / 32 output rows each)

_cache = {}


def _build_program():
    import concourse.bacc as bacc
    import concourse.mybir as mybir
    from concourse.tile import TileContext

    F32 = mybir.dt.float32
    F16 = mybir.dt.float16
    nc = bacc.Bacc("TRN2", target_bir_lowering=False, debug=False, num_devices=1)

    crd = nc.dram_tensor("crd", [BPC, D, NP], F16, kind="ExternalInput")
    a4d = nc.dram_tensor("a4d", [BPC, D, NT], F32, kind="ExternalInput")
    wpd = nc.dram_tensor("wpd", [D, 132], F16, kind="ExternalInput")
    b2r = nc.dram_tensor("b2r", [D, 1], F32, kind="ExternalInput")
    raw = nc.dram_tensor("raw", [BPC, NSG, 100, 2 * NP], F16, kind="ExternalOutput")

    add = mybir.AluOpType.add
    amax = mybir.AluOpType.max
    Relu = mybir.ActivationFunctionType.Relu

    with TileContext(nc) as tc:
        with (
            tc.tile_pool(name="const", bufs=1) as cpool,
            tc.tile_pool(name="batch", bufs=2) as bpool,
            tc.tile_pool(name="x", bufs=12) as xpool,
            tc.tile_pool(name="y", bufs=8) as ypool,
            tc.tile_pool(name="o", bufs=3) as opool,
            tc.tile_pool(name="pz", bufs=2, space="PSUM") as pzpool,
            tc.tile_pool(name="po", bufs=2, space="PSUM") as popool,
        ):
            # inputs for batch 0 first: they gate the first X / fc2
            creps = []
            a4s = []
            for b in range(BPC):
                crep = bpool.tile([D, NP], F16, tag="crep")
                nc.sync.dma_start(crep[:], crd.ap()[b, :, :])
                creps.append(crep)
                a4 = bpool.tile([D, NT], F32, tag="a4")
                nc.sync.dma_start(a4[:], a4d.ap()[b, :, :])
                a4s.append(a4)
                if b == 0:
                    wpt = cpool.tile([D, 132], F16)
                    nc.sync.dma_start(wpt[:], wpd.ap()[:, :])
                    w2t = wpt[:, 0:128]
                    w3t = wpt[:, 128:132]
                    b2t = cpool.tile([D, 1], F32)
                    nc.sync.dma_start(b2t[:], b2r.ap()[:, :])

            for b in range(BPC):
                crep = creps[b]
                a4 = a4s[b]

                # main pair loop, software-pipelined with per-stage skews
                pend_x = {}
                pend_pz = {}
                pend_y = {}
                pend_po = {}
                po2 = None
                for p in range(PAIRS + 3 + 5):
                    if p < PAIRS:
                        t0 = 2 * p
                        xs = []
                        for t in (t0, t0 + 1):
                            x = xpool.tile([D, NP], F16, tag="x")
                            nc.vector.tensor_scalar(
                                out=x[:],
                                in0=crep[:],
                                scalar1=a4[:, t : t + 1],
                                scalar2=0.0,
                                op0=add,
                                op1=amax,
                            )
                            xs.append(x)
                        pend_x[p] = xs

                    pm = p - 1
                    if 0 <= pm < PAIRS:
                        xs = pend_x.pop(pm)
                        pz2 = pzpool.tile([D, 2 * NP], F32, tag="pz")
                        nc.tensor.matmul(
                            pz2[:, 0:NP], w2t, xs[0][:], start=True, stop=True
                        )
                        nc.tensor.matmul(
                            pz2[:, NP : 2 * NP], w2t, xs[1][:], start=True, stop=True
                        )
                        pend_pz[pm] = pz2

                    py = p - 2
                    if 0 <= py < PAIRS:
                        pz2 = pend_pz.pop(py)
                        y2 = ypool.tile([D, 2 * NP], F16, tag="y2")
                        nc.scalar.activation(y2[:], pz2[:], Relu, bias=b2t[:, 0:1])
                        pend_y[py] = y2

                    pf = p - 3
                    if 0 <= pf < PAIRS:
                        s, q = divmod(pf, 4)
                        if q == 0:
                            po2 = popool.tile([D, 2 * NP], F32, tag="po")
                        y2 = pend_y.pop(pf)
                        for j in (0, 1):
                            st = 2 * q + j
                            h, v = st // 4, st % 4
                            nc.tensor.matmul(
                                po2[32 * v : 32 * v + 4, h * NP : h * NP + NP],
                                w3t,
                                y2[:, j * NP : j * NP + NP],
                                start=True,
                                stop=True,
                                tile_position=(0, 32 * v),
                            )
                        if pf == PAIRS - 1 and b == BPC - 1:
                            # very last supergroup: shallow copy + trimmed
                            # split dump so the end-of-kernel drain is short
                            ob2 = opool.tile([D, 2 * NP], F16, tag="ob")
                            nc.vector.tensor_scalar_add(
                                ob2[0:100, :], po2[0:100, :], 0.0
                            )
                            nc.sync.dma_start(
                                raw.ap()[b, s, 0:100, 0:NP], ob2[0:100, 0:NP]
                            )
                            nc.sync.dma_start(
                                raw.ap()[b, s, 0:40, NP : 2 * NP],
                                ob2[0:40, NP : 2 * NP],
                            )
                        elif q == 3 or pf == PAIRS - 1:
                            pend_po[s] = po2

                    # copy/DMA two iters after a supergroup's last fc3, so the
                    # DVE copy never waits mid-queue and never delays X ops
                    pc = p - 8
                    if pc >= 0 and pc % 4 == 0 and (pc // 4) in pend_po:
                        s2 = pc // 4
                        po2c = pend_po.pop(s2)
                        # used fc3 partitions are {32v+r, r<4} (v = slot%4,
                        # max 99); dump them raw, host un-permutes rows.
                        ob2 = opool.tile([D, 2 * NP], F16, tag="ob")
                        nc.vector.tensor_scalar_add(ob2[0:100, :], po2c[0:100, :], 0.0)
                        nc.sync.dma_start(raw.ap()[b, s2, :, :], ob2[0:100, :])

    nc.compile()
    return nc


def _host_prep(h_hat, pos_pickup, pos_delivery, solution, Wq1, Wk1, Wq2, Wk2,
               fc1_w, fc1_b):
    """Host-side A/C maps folded with fc1: returns crep (f16) and a4 (f32)."""
    f32 = np.float32
    h_hat = np.asarray(h_hat, f32)
    pp = np.asarray(pos_pickup).astype(np.int64)
    pd = np.asarray(pos_delivery).astype(np.int64)
    sol = np.asarray(solution).astype(np.int64)
    Wq1 = np.asarray(Wq1, f32)
    Wk1 = np.asarray(Wk1, f32)
    Wq2 = np.asarray(Wq2, f32)
    Wk2 = np.asarray(Wk2, f32)
    fc1_w = np.asarray(fc1_w, f32)
    fc1_b = np.asarray(fc1_b, f32)

    crep = np.zeros((B, D, NP), np.float16)
    a4 = np.zeros((B, D, NT), f32)

    for b in range(B):
        hb = h_hat[b]  # (N, D)
        hnb = hb[sol[b]]  # (N, D) gathered neighbours
        p = hb[pp[b]]  # (D,)
        dv = hb[pd[b]]
        # u[h] = Wk[h] @ (Wq[h]^T @ q): compat[n,h] = x[n] . u[h]
        U1p = np.stack([Wk1[h] @ (Wq1[h].T @ p) for h in range(H)], axis=1)
        U2p = np.stack([Wk2[h] @ (Wq2[h].T @ p) for h in range(H)], axis=1)
        U1d = np.stack([Wk1[h] @ (Wq1[h].T @ dv) for h in range(H)], axis=1)
        U2d = np.stack([Wk2[h] @ (Wq2[h].T @ dv) for h in range(H)], axis=1)
        A = hb @ (U1p @ fc1_w[0:4]) + hnb @ (U2p @ fc1_w[4:8])  # (N, 32)
        C = hb @ (U1d @ fc1_w[8:12]) + hnb @ (U2d @ fc1_w[12:16])  # (N, 32)
        Cp = np.zeros((NP, 32), f32)
        Cp[:N] = C
        crep[b] = np.tile((Cp + fc1_b).T.astype(np.float16), (4, 1))
        Ap = np.zeros((4 * NT, 32), f32)
        Ap[:N] = A
        # a4[32r+k, t] = A[4t+r, k]
        a4[b] = Ap.reshape(NT, 4, 32).transpose(1, 2, 0).reshape(D, NT)
    return crep, a4


_last_results = None


def _ensure_axon_hooks():
    """bass_utils' axon trace path hard-imports antenv.axon_hooks; provide a
    stub (and wire the real NTFF hook when available) if it's absent, so a
    BASS_TRACE=1 environment without the module doesn't crash the kernel."""
    import sys
    import types

    try:
        import antenv.axon_hooks  # noqa: F401
        return
    except ImportError:
        pass
    try:
        import antenv
    except ImportError:
        return
    mod = types.ModuleType("antenv.axon_hooks")
    state = {"hook": None}
    mod.set_axon_ntff_profile_hook = lambda hook: state.__setitem__("hook", hook)
    mod.get_axon_ntff_profile_hook = lambda: state["hook"]
    sys.modules["antenv.axon_hooks"] = mod
    antenv.axon_hooks = mod
    try:
        from trn_agent_boot.trn_boot import _ntff_profile_via_ctypes

        mod.set_axon_ntff_profile_hook(
            _ntff_profile_via_ctypes("/opt/axon/libaxon_pjrt.so")
        )
    except Exception:
        pass


def kernel(
    h_hat,
    pos_pickup,
    pos_delivery,
    solution,
    Wq1,
    Wk1,
    Wq2,
    Wk2,
    fc1_w,
    fc1_b,
    fc2_w,
    fc2_b,
    fc3_w,
    fc3_b,
):
    global _last_results
    _ensure_axon_hooks()
    from concourse.bass_utils import run_bass_kernel_spmd

    f32 = np.float32
    fc2_w = np.asarray(fc2_w, f32)
    fc2_b = np.asarray(fc2_b, f32)
    fc3_w = np.asarray(fc3_w, f32)
    fc3_b = np.asarray(fc3_b, f32)

    crep, a4 = _host_prep(
        h_hat, pos_pickup, pos_delivery, solution, Wq1, Wk1, Wq2, Wk2,
        np.asarray(fc1_w, f32), np.asarray(fc1_b, f32),
    )

    # block-diagonal packed MLP weights (4 independent 32-blocks)
    w2d = np.zeros((D, 128), f32)
    w3d = np.zeros((D, 4), f32)
    for r in range(4):
        w2d[32 * r : 32 * r + 32, 32 * r : 32 * r + 32] = fc2_w
        w3d[32 * r : 32 * r + 32, r : r + 1] = fc3_w.reshape(32, 1)
    b2r = np.tile(fc2_b.reshape(32, 1), (4, 1)).astype(f32)
    wp = np.concatenate([w2d, w3d], axis=1).astype(np.float16)  # [D, 132]

    if "nc" not in _cache:
        _cache["nc"] = _build_program()
    nc = _cache["nc"]

    in_maps = []
    for c in range(NCORES):
        bs = slice(BPC * c, BPC * (c + 1))
        in_maps.append(
            {
                "crd": np.ascontiguousarray(crep[bs]),
                "a4d": np.ascontiguousarray(a4[bs]),
                "wpd": wp,
                "b2r": b2r,
            }
        )

    res = run_bass_kernel_spmd(nc, in_maps, core_ids=list(range(NCORES)))
    _last_results = res

    # un-permute: raw[b, s, 32v+r, 512h+j] holds out row 32s+16h+4v+r
    rows = np.arange(N)
    s_i = rows // 32
    rem = rows % 32
    h_i = rem // 16
    rem2 = rem % 16
    part = 32 * (rem2 // 4) + (rem2 % 4)
    foff = NP * h_i
    cols = np.arange(N)
    out = np.empty((B, N, N), f32)
    for c in range(NCORES):
        rawc = res.results[c]["raw"].astype(f32)  # [BPC, NSG, 100, 2*NP]
        for bb in range(BPC):
            out[BPC * c + bb] = rawc[
                bb, s_i[:, None], part[:, None], foff[:, None] + cols[None, :]
            ]
    b3 = float(fc3_b.reshape(-1)[0])
    if b3 != 0.0:
        out = out + b3
    return out.astype(f32)


# revision 17
# speedup vs baseline: 1.0171x; 1.0079x over previous
"""Trainium2 Bass kernel for nn_NNSDecoder (gnn_message_passing).

Reference computation (B=16, N=501, D=128, H=4):
    out[b,i,j] = fc3 . relu(fc2^T relu(feat @ fc1 + b1) + b2) + b3
    feat[b,i,j] = [cp_pre[b,i], cp_post[b,i], cd_pre[b,j], cd_post[b,j]]  (4H=16)

Key algebra: compat[b,n,h] = x[b,n] . (Wk[h] Wq[h]^T q_b), so every
pickup/delivery-side term is linear in h_hat / h_nb rows.  Folding the
head projections and fc1 together gives per-batch maps
    A[b] = h_hat[b] @ G_A1 + h_nb[b] @ G_A2          (N x 32, row/i term)
    C[b] = h_hat[b] @ G_C1 + h_nb[b] @ G_C2          (N x 32, col/j term)
    out[b,i,j] = w3 . relu(W2^T relu(A[b,i] + C[b,j] + b1) + b2) + b3
A and C are tiny (N x 32) and are computed on HOST in fp32; the device
receives crep = (C+b1) replicated 4x across partitions (f16) and
a4[32r+k, t] = A[4t+r, k] (f32), and does only the O(N^2) work.

Per 4-row i-tile t:
    X_t = relu(crep + a4[:,t])            (DVE tensor_scalar, f16, 2x mode)
    Z_t = W2blk @ X_t                     (PE matmul, block-diag f16)
    Y_t = relu(Z_t + b2)                  (ScalarE ACT, f16)
    po  = w3blk @ Y_t                     (PE matmul into packed PSUM)
i-tiles are processed in PAIRS sharing one 2-bank PSUM tile [128,1024]
(NP=512 = exact bank) so Y runs as a single wide op; the stages are
software-pipelined with per-stage skews (X@i, fc2@i-1, Y@i-2, fc3@i-3,
PSUM->SBUF copy@deep skew on DVE) so no engine's in-order queue waits
on another engine's freshest output.  8-tile supergroups drain with one
wide f16 copy + one contiguous raw-dump DMA; the host un-permutes rows.

Sharding: batch dim 16 -> 8 cores x 2 batches (data parallel, weights
replicated). Full inputs in, full output out.
"""

import numpy as np

B, N, D, H = 16, 501, 128, 4
NCORES = 8
BPC = B // NCORES  # batches per core
NP = 512  # padded j: exact PSUM bank (512 f32 = 2KB)
NT = 126  # i-tiles of 4 rows (126*4 = 504 >= 501)
PAIRS = NT // 2  # 63 i-tile pairs per batch
NJ = 504  # streamed j columns per tile (504 >= 501; banks stay 512-aligned)
NSG = 16  # supergroups per batch (8 i-tiles / 32 output rows each)

_cache = {}


def _build_program():
    import concourse.bacc as bacc
    import concourse.mybir as mybir
    from concourse.tile import TileContext

    F32 = mybir.dt.float32
    F16 = mybir.dt.float16
    nc = bacc.Bacc("TRN2", target_bir_lowering=False, debug=False, num_devices=1)

    crd = nc.dram_tensor("crd", [BPC, D, NP], F16, kind="ExternalInput")
    a4d = nc.dram_tensor("a4d", [BPC, D, NT], F32, kind="ExternalInput")
    wpd = nc.dram_tensor("wpd", [D, 132], F16, kind="ExternalInput")
    b2r = nc.dram_tensor("b2r", [D, 1], F32, kind="ExternalInput")
    raw = nc.dram_tensor("raw", [BPC, NSG, 100, 2 * NP], F16, kind="ExternalOutput")

    add = mybir.AluOpType.add
    amax = mybir.AluOpType.max
    Relu = mybir.ActivationFunctionType.Relu

    with TileContext(nc) as tc:
        with (
            tc.tile_pool(name="const", bufs=1) as cpool,
            tc.tile_pool(name="batch", bufs=2) as bpool,
            tc.tile_pool(name="x", bufs=12) as xpool,
            tc.tile_pool(name="y", bufs=8) as ypool,
            tc.tile_pool(name="o", bufs=3) as opool,
            tc.tile_pool(name="pz", bufs=2, space="PSUM") as pzpool,
            tc.tile_pool(name="po", bufs=2, space="PSUM") as popool,
        ):
            # inputs for batch 0 first: they gate the first X / fc2
            creps = []
            a4s = []
            for b in range(BPC):
                crep = bpool.tile([D, NP], F16, tag="crep")
                nc.sync.dma_start(crep[:], crd.ap()[b, :, :])
                creps.append(crep)
                a4 = bpool.tile([D, NT], F32, tag="a4")
                nc.sync.dma_start(a4[:], a4d.ap()[b, :, :])
                a4s.append(a4)
                if b == 0:
                    wpt = cpool.tile([D, 132], F16)
                    nc.sync.dma_start(wpt[:], wpd.ap()[:, :])
                    w2t = wpt[:, 0:128]
                    w3t = wpt[:, 128:132]
                    b2t = cpool.tile([D, 1], F32)
                    nc.sync.dma_start(b2t[:], b2r.ap()[:, :])

            for b in range(BPC):
                crep = creps[b]
                a4 = a4s[b]

                # main pair loop, software-pipelined with per-stage skews
                pend_x = {}
                pend_pz = {}
                pend_y = {}
                pend_po = {}
                po2 = None
                for p in range(PAIRS + 3 + 5):
                    if p < PAIRS:
                        t0 = 2 * p
                        xs = []
                        for t in (t0, t0 + 1):
                            x = xpool.tile([D, NP], F16, tag="x")
                            nc.vector.tensor_scalar(
                                out=x[:, 0:NJ],
                                in0=crep[:, 0:NJ],
                                scalar1=a4[:, t : t + 1],
                                scalar2=0.0,
                                op0=add,
                                op1=amax,
                            )
                            xs.append(x)
                        pend_x[p] = xs

                    pm = p - 1
                    if 0 <= pm < PAIRS:
                        xs = pend_x.pop(pm)
                        pz2 = pzpool.tile([D, 2 * NP], F32, tag="pz")
                        nc.tensor.matmul(
                            pz2[:, 0:NJ], w2t, xs[0][:, 0:NJ], start=True, stop=True
                        )
                        nc.tensor.matmul(
                            pz2[:, NP : NP + NJ], w2t, xs[1][:, 0:NJ],
                            start=True, stop=True,
                        )
                        pend_pz[pm] = pz2

                    py = p - 2
                    if 0 <= py < PAIRS:
                        pz2 = pend_pz.pop(py)
                        y2 = ypool.tile([D, 2 * NP], F16, tag="y2")
                        nc.scalar.activation(
                            y2[:, 0 : NP + NJ], pz2[:, 0 : NP + NJ], Relu,
                            bias=b2t[:, 0:1],
                        )
                        pend_y[py] = y2

                    pf = p - 3
                    if 0 <= pf < PAIRS:
                        s, q = divmod(pf, 4)
                        if q == 0:
                            po2 = popool.tile([D, 2 * NP], F32, tag="po")
                        y2 = pend_y.pop(pf)
                        for j in (0, 1):
                            st = 2 * q + j
                            h, v = st // 4, st % 4
                            nc.tensor.matmul(
                                po2[32 * v : 32 * v + 4, h * NP : h * NP + NJ],
                                w3t,
                                y2[:, j * NP : j * NP + NJ],
                                start=True,
                                stop=True,
                                tile_position=(0, 32 * v),
                            )
                        last_sg = b == BPC - 1 and s == NSG - 1
                        if last_sg and pf == PAIRS - 2:
                            # final supergroup, h=0 rows complete: drain early
                            ob2 = opool.tile([D, 2 * NP], F16, tag="ob")
                            nc.vector.tensor_scalar_add(
                                ob2[0:100, 0:NJ], po2[0:100, 0:NJ], 0.0
                            )
                            nc.sync.dma_start(
                                raw.ap()[b, s, 0:100, 0:NP], ob2[0:100, 0:NP]
                            )
                        elif last_sg and pf == PAIRS - 1:
                            # h=1 remainder (only rows 496-500 are real)
                            ob2 = opool.tile([D, 2 * NP], F16, tag="ob")
                            nc.vector.tensor_scalar_add(
                                ob2[0:40, NP : NP + NJ],
                                po2[0:40, NP : NP + NJ],
                                0.0,
                            )
                            nc.sync.dma_start(
                                raw.ap()[b, s, 0:40, NP : 2 * NP],
                                ob2[0:40, NP : 2 * NP],
                            )
                        elif q == 3 or pf == PAIRS - 1:
                            pend_po[s] = po2

                    # copy/DMA two iters after a supergroup's last fc3, so the
                    # DVE copy never waits mid-queue and never delays X ops
                    pc = p - 8
                    if pc >= 0 and pc % 4 == 0 and (pc // 4) in pend_po:
                        s2 = pc // 4
                        po2c = pend_po.pop(s2)
                        # used fc3 partitions are {32v+r, r<4} (v = slot%4,
                        # max 99); dump them raw, host un-permutes rows.
                        ob2 = opool.tile([D, 2 * NP], F16, tag="ob")
                        nc.vector.tensor_scalar_add(
                            ob2[0:100, 0 : NP + NJ], po2c[0:100, 0 : NP + NJ], 0.0
                        )
                        nc.sync.dma_start(raw.ap()[b, s2, :, :], ob2[0:100, :])

    nc.compile()
    return nc


def _host_prep(h_hat, pos_pickup, pos_delivery, solution, Wq1, Wk1, Wq2, Wk2,
               fc1_w, fc1_b):
    """Host-side A/C maps folded with fc1: returns crep (f16) and a4 (f32)."""
    f32 = np.float32
    h_hat = np.asarray(h_hat, f32)
    pp = np.asarray(pos_pickup).astype(np.int64)
    pd = np.asarray(pos_delivery).astype(np.int64)
    sol = np.asarray(solution).astype(np.int64)
    Wq1 = np.asarray(Wq1, f32)
    Wk1 = np.asarray(Wk1, f32)
    Wq2 = np.asarray(Wq2, f32)
    Wk2 = np.asarray(Wk2, f32)
    fc1_w = np.asarray(fc1_w, f32)
    fc1_b = np.asarray(fc1_b, f32)

    crep = np.zeros((B, D, NP), np.float16)
    a4 = np.zeros((B, D, NT), f32)

    for b in range(B):
        hb = h_hat[b]  # (N, D)
        hnb = hb[sol[b]]  # (N, D) gathered neighbours
        p = hb[pp[b]]  # (D,)
        dv = hb[pd[b]]
        # u[h] = Wk[h] @ (Wq[h]^T @ q): compat[n,h] = x[n] . u[h]
        U1p = np.stack([Wk1[h] @ (Wq1[h].T @ p) for h in range(H)], axis=1)
        U2p = np.stack([Wk2[h] @ (Wq2[h].T @ p) for h in range(H)], axis=1)
        U1d = np.stack([Wk1[h] @ (Wq1[h].T @ dv) for h in range(H)], axis=1)
        U2d = np.stack([Wk2[h] @ (Wq2[h].T @ dv) for h in range(H)], axis=1)
        A = hb @ (U1p @ fc1_w[0:4]) + hnb @ (U2p @ fc1_w[4:8])  # (N, 32)
        C = hb @ (U1d @ fc1_w[8:12]) + hnb @ (U2d @ fc1_w[12:16])  # (N, 32)
        Cp = np.zeros((NP, 32), f32)
        Cp[:N] = C
        crep[b] = np.tile((Cp + fc1_b).T.astype(np.float16), (4, 1))
        Ap = np.zeros((4 * NT, 32), f32)
        Ap[:N] = A
        # a4[32r+k, t] = A[4t+r, k]
        a4[b] = Ap.reshape(NT, 4, 32).transpose(1, 2, 0).reshape(D, NT)
    return crep, a4


_last_results = None


def _ensure_axon_hooks():
    """bass_utils' axon trace path hard-imports antenv.axon_hooks; provide a
    stub (and wire the real NTFF hook when available) if it's absent, so a
    BASS_TRACE=1 environment without the module doesn't crash the kernel."""
    import sys
    import types

    try:
        import antenv.axon_hooks  # noqa: F401
        return
    except ImportError:
        pass
    try:
        import antenv
    except ImportError:
        return
    mod = types.ModuleType("antenv.axon_hooks")
    state = {"hook": None}
    mod.set_axon_ntff_profile_hook = lambda hook: state.__setitem__("hook", hook)
    mod.get_axon_ntff_profile_hook = lambda: state["hook"]
    sys.modules["antenv.axon_hooks"] = mod
    antenv.axon_hooks = mod
    try:
        from trn_agent_boot.trn_boot import _ntff_profile_via_ctypes

        mod.set_axon_ntff_profile_hook(
            _ntff_profile_via_ctypes("/opt/axon/libaxon_pjrt.so")
        )
    except Exception:
        pass


def kernel(
    h_hat,
    pos_pickup,
    pos_delivery,
    solution,
    Wq1,
    Wk1,
    Wq2,
    Wk2,
    fc1_w,
    fc1_b,
    fc2_w,
    fc2_b,
    fc3_w,
    fc3_b,
):
    global _last_results
    _ensure_axon_hooks()
    from concourse.bass_utils import run_bass_kernel_spmd

    f32 = np.float32
    fc2_w = np.asarray(fc2_w, f32)
    fc2_b = np.asarray(fc2_b, f32)
    fc3_w = np.asarray(fc3_w, f32)
    fc3_b = np.asarray(fc3_b, f32)

    crep, a4 = _host_prep(
        h_hat, pos_pickup, pos_delivery, solution, Wq1, Wk1, Wq2, Wk2,
        np.asarray(fc1_w, f32), np.asarray(fc1_b, f32),
    )

    # block-diagonal packed MLP weights (4 independent 32-blocks)
    w2d = np.zeros((D, 128), f32)
    w3d = np.zeros((D, 4), f32)
    for r in range(4):
        w2d[32 * r : 32 * r + 32, 32 * r : 32 * r + 32] = fc2_w
        w3d[32 * r : 32 * r + 32, r : r + 1] = fc3_w.reshape(32, 1)
    b2r = np.tile(fc2_b.reshape(32, 1), (4, 1)).astype(f32)
    wp = np.concatenate([w2d, w3d], axis=1).astype(np.float16)  # [D, 132]

    if "nc" not in _cache:
        _cache["nc"] = _build_program()
    nc = _cache["nc"]

    in_maps = []
    for c in range(NCORES):
        bs = slice(BPC * c, BPC * (c + 1))
        in_maps.append(
            {
                "crd": np.ascontiguousarray(crep[bs]),
                "a4d": np.ascontiguousarray(a4[bs]),
                "wpd": wp,
                "b2r": b2r,
            }
        )

    res = run_bass_kernel_spmd(nc, in_maps, core_ids=list(range(NCORES)))
    _last_results = res

    # un-permute: raw[b, s, 32v+r, 512h+j] holds out row 32s+16h+4v+r
    rows = np.arange(N)
    s_i = rows // 32
    rem = rows % 32
    h_i = rem // 16
    rem2 = rem % 16
    part = 32 * (rem2 // 4) + (rem2 % 4)
    foff = NP * h_i
    cols = np.arange(N)
    out = np.empty((B, N, N), f32)
    for c in range(NCORES):
        rawc = res.results[c]["raw"].astype(f32)  # [BPC, NSG, 100, 2*NP]
        for bb in range(BPC):
            out[BPC * c + bb] = rawc[
                bb, s_i[:, None], part[:, None], foff[:, None] + cols[None, :]
            ]
    b3 = float(fc3_b.reshape(-1)[0])
    if b3 != 0.0:
        out = out + b3
    return out.astype(f32)
